# revision 14
# baseline (speedup 1.0000x reference)
"""Trainium2 Bass kernel for nn_GCNModel (6-layer GCN + 3-layer FC mesh deformer).

Strategy
--------
Data-parallel over batch B=32 across 8 NeuronCores (4 batch elements each).

Algebraic restructuring (host side, exact):
  ReLU only follows GCN layers 2, 4, 6, so each pair of GCN layers collapses:
      A(A x W1 + 1 b1^T) W2 + 1 b2^T
        = A^2 x (W1 W2) + (A 1) (b1 W2)^T + 1 b2^T
  with A the dense-ified normalized adjacency.  Three aggregations with a
  host-precomputed dense A^2 replace six sparse gather/scatter aggregations.
  Further:
    * pair 1's aggregation input is rank-3 (x = [verts | 1 img^T]):
      A^2 x W12 = (A^2 verts) W12[:3] + (A^2 1) (img W12[3:])^T
      so the wide aggregation reduces to a width-3 one plus rank-1 terms,
      all folded into a single rank-6 matmul.
    * pair 3 aggregates after the [512,3] transform (width 3).
  Only pair 2 needs a full width-512 dense A^2 apply per batch element.

On device everything runs in bf16 operands with fp32 PSUM accumulation
(validated vs fp32 reference on host: ~1.7e-4 max relative error; the output
is dominated by `vertices` plus a 0.1-scaled tanh-squashed deformation).

Layouts alternate vertex-major / feature-major so no transposes are needed
in the hot path:
  agg (contracts over vertices):  lhsT = t (vertex-major), rhs = A2T rows
                                  -> feature-major output
  transform (contracts over features): lhsT = x (feature-major), rhs = W
                                  -> vertex-major output
"""

import numpy as np
import ml_dtypes

B, V, E, IMG_F = 32, 2048, 12288, 512
N_CORES = 8
BL = B // N_CORES  # 4 batch elements per core
P = 128
NV = V // P   # 16 vertex chunks
F = 512
NF = F // P   # 4 feature chunks
FC_H = 1024
FLAT = V * 3  # 6144
NKFC1 = FLAT // P  # 48
NKFC2 = FC_H // P  # 8

BF16 = ml_dtypes.bfloat16

_CACHE = {}


def _host_prep(inputs):
    """Exact (fp64) host-side algebra: dense A^2, collapsed weights, shards."""
    ei = np.asarray(inputs["edge_index"])
    src = np.concatenate([ei[0], np.arange(V)]).astype(np.int64)
    dst = np.concatenate([ei[1], np.arange(V)]).astype(np.int64)
    deg = np.zeros(V)
    np.add.at(deg, dst, 1.0)
    dinv = 1.0 / np.sqrt(deg)
    normv = dinv[src] * dinv[dst]
    A = np.zeros((V, V))
    np.add.at(A, (dst, src), normv)
    A2 = A @ A
    rho = (A @ np.ones(V)).astype(np.float32)
    rho2 = (A2 @ np.ones(V)).astype(np.float32)

    W = [np.asarray(inputs[f"W{i}"], np.float64) for i in range(1, 7)]
    bb = [np.asarray(inputs[f"b{i}"], np.float64) for i in range(1, 7)]
    W12 = W[0] @ W[1]
    W34 = W[2] @ W[3]
    W56 = W[4] @ W[5]
    bias1 = bb[0] @ W[1]  # pairs with rho
    bias2 = bb[2] @ W[3]
    bias3 = bb[4] @ W[5]
    b2, b4, b6 = bb[1], bb[3], bb[5]

    def pack_rows(w, ncol):
        # [nk*128, ncol] -> [128, nk*ncol] with chunk kc at cols [kc*ncol:...]
        w = np.asarray(w, np.float32)
        nk = w.shape[0] // P
        return np.ascontiguousarray(
            w.reshape(nk, P, ncol).transpose(1, 0, 2).reshape(P, nk * ncol)
        )

    shared = {}
    shared["A2T"] = np.ascontiguousarray(A2.T).astype(BF16)
    # k=3 static lhsT for pair1 (verts rows); the per-batch image term
    # c1 (x) rho2 is a separate k=1 matmul, biases a separate k=2 matmul.
    shared["W12A"] = np.asarray(W12[:3], np.float32).astype(BF16)
    bias_pack1 = np.stack([bias1, b2]).astype(np.float32)  # pairs with rho1
    shared["HAS_BIAS1"] = bool(np.any(bias_pack1))
    shared["BIASP1"] = bias_pack1.astype(BF16)
    shared["RHO2"] = rho2.reshape(1, V).astype(BF16)
    shared["RHO1"] = np.stack([rho, np.ones(V, np.float32)]).astype(BF16)
    shared["W12B"] = pack_rows(W12[3:], F).astype(BF16)
    shared["W34"] = pack_rows(W34, F).astype(BF16)
    shared["W56"] = pack_rows(W56, 3).astype(BF16)

    # pair2/3 bias packs (zero in the shipped model; matmul-folded if not)
    bias_pack2 = np.stack([bias2, b4]).astype(np.float32)  # [2, 512]
    bias_pack3 = np.zeros((2, BL * 3), np.float32)
    for b in range(BL):
        bias_pack3[0, b * 3:(b + 1) * 3] = bias3
        bias_pack3[1, b * 3:(b + 1) * 3] = b6
    shared["HAS_BIAS2"] = bool(np.any(bias_pack2))
    shared["HAS_BIAS3"] = bool(np.any(bias_pack3))
    shared["BIASP2"] = bias_pack2.astype(BF16)
    shared["BIASP3"] = bias_pack3.astype(BF16)

    # FC weights, bf16. fcW1 rows permuted: new row (vc*3+c)*128+p
    # corresponds to original row (vc*128+p)*3+c.
    fcW1 = np.asarray(inputs["fcW1"], np.float32)
    idx = (
        (np.arange(NV)[:, None, None] * P + np.arange(P)[None, None, :]) * 3
        + np.arange(3)[None, :, None]
    ).reshape(-1)  # (vc, c, p) -> orig row
    shared["FCW1"] = np.ascontiguousarray(fcW1[idx]).astype(BF16)
    shared["FCW2"] = np.asarray(inputs["fcW2"], np.float32).astype(BF16)
    shared["FCW3"] = np.asarray(inputs["fcW3"], np.float32).astype(BF16)
    fcb1 = np.asarray(inputs["fcb1"], np.float32)
    fcb2 = np.asarray(inputs["fcb2"], np.float32)
    fcb3 = np.asarray(inputs["fcb3"], np.float32)
    shared["HAS_FCB"] = bool(np.any(fcb1) or np.any(fcb2) or np.any(fcb3))
    shared["FCB1"] = np.ascontiguousarray(np.broadcast_to(fcb1, (BL, FC_H)))
    shared["FCB2"] = np.ascontiguousarray(np.broadcast_to(fcb2, (BL, FC_H)))
    shared["FCB3"] = np.ascontiguousarray(np.broadcast_to(fcb3, (BL, FLAT)))

    # per-core shards
    verts = np.asarray(inputs["vertices"], np.float32)  # [B, V, 3]
    img = np.asarray(inputs["img_features"], np.float32)  # [B, 512]
    per_core = []
    for c in range(N_CORES):
        vb = verts[c * BL:(c + 1) * BL]  # [BL, V, 3]
        # vertex-major agg rhs: [uc, p, b*3+cc] = verts[b, uc*128+p, cc]
        vvm = np.ascontiguousarray(
            vb.transpose(1, 0, 2).reshape(NV, P, BL * 3)
        ).astype(BF16)
        per_core.append({
            "VVM": vvm,
            "VFLAT": np.ascontiguousarray(vb.reshape(BL, FLAT)),
            "IMG": np.ascontiguousarray(img[c * BL:(c + 1) * BL]).astype(BF16),
        })
    return shared, per_core


def _build_program(has_bias1, has_bias2, has_bias3, has_fcb):
    """Emit the Bass/Tile program (identical on all cores)."""
    from concourse import bacc, bass, mybir, tile
    from concourse.masks import make_identity

    f32 = mybir.dt.float32
    bf16 = mybir.dt.bfloat16
    AF = mybir.ActivationFunctionType

    nc = bacc.Bacc(trn_type="TRN2")

    d_a2t = nc.dram_tensor("A2T", [V, V], bf16, kind="ExternalInput")
    d_w12a = nc.dram_tensor("W12A", [3, F], bf16, kind="ExternalInput")
    d_biasp1 = nc.dram_tensor("BIASP1", [2, F], bf16, kind="ExternalInput")
    d_rho2 = nc.dram_tensor("RHO2", [1, V], bf16, kind="ExternalInput")
    d_rho1 = nc.dram_tensor("RHO1", [2, V], bf16, kind="ExternalInput")
    d_w12b = nc.dram_tensor("W12B", [P, 4 * F], bf16, kind="ExternalInput")
    d_w34 = nc.dram_tensor("W34", [P, 4 * F], bf16, kind="ExternalInput")
    d_w56 = nc.dram_tensor("W56", [P, 12], bf16, kind="ExternalInput")
    d_biasp2 = nc.dram_tensor("BIASP2", [2, F], bf16, kind="ExternalInput")
    d_biasp3 = nc.dram_tensor("BIASP3", [2, BL * 3], bf16, kind="ExternalInput")
    d_fcw1 = nc.dram_tensor("FCW1", [FLAT, FC_H], bf16, kind="ExternalInput")
    d_fcw2 = nc.dram_tensor("FCW2", [FC_H, FC_H], bf16, kind="ExternalInput")
    d_fcw3 = nc.dram_tensor("FCW3", [FC_H, FLAT], bf16, kind="ExternalInput")
    d_fcb1 = nc.dram_tensor("FCB1", [BL, FC_H], f32, kind="ExternalInput")
    d_fcb2 = nc.dram_tensor("FCB2", [BL, FC_H], f32, kind="ExternalInput")
    d_fcb3 = nc.dram_tensor("FCB3", [BL, FLAT], f32, kind="ExternalInput")
    d_vvm = nc.dram_tensor("VVM", [NV, P, BL * 3], bf16, kind="ExternalInput")
    d_vflat = nc.dram_tensor("VFLAT", [BL, FLAT], f32, kind="ExternalInput")
    d_img = nc.dram_tensor("IMG", [BL, IMG_F], bf16, kind="ExternalInput")
    d_out = nc.dram_tensor("OUT", [BL, FLAT], f32, kind="ExternalOutput")

    G = BL * 3  # 12: per-vertex-chunk group width (batch x coord)

    with tile.TileContext(nc) as tc:
        with (
            tc.tile_pool(name="const", bufs=1) as const_pool,
            tc.tile_pool(name="x", bufs=4) as x_pool,
            tc.tile_pool(name="tbf", bufs=1) as tbf_pool,
            tc.tile_pool(name="work", bufs=4) as work_pool,
            tc.tile_pool(name="stream", bufs=8) as stream_pool,
            tc.tile_pool(name="hfin", bufs=4) as hfin_pool,
            tc.tile_pool(name="psA", bufs=3, space="PSUM") as psA,
            tc.tile_pool(name="psB", bufs=2, space="PSUM") as psB,
        ):
            # ---------- resident constants ----------
            a2t = []
            for uc in range(NV):
                t = const_pool.tile([P, V], bf16, tag=f"a2t{uc}")
                nc.sync.dma_start(out=t[:], in_=d_a2t[uc * P:(uc + 1) * P, :])
                a2t.append(t)

            w12a_base = const_pool.tile([3, F], bf16, tag="w12a")
            nc.sync.dma_start(out=w12a_base[:], in_=d_w12a[:])
            if has_bias1:
                biasp1 = const_pool.tile([2, F], bf16, tag="biasp1")
                nc.sync.dma_start(out=biasp1[:], in_=d_biasp1[:])
            rho2_sb = const_pool.tile([1, V], bf16, tag="rho2")
            nc.sync.dma_start(out=rho2_sb[:], in_=d_rho2[:])
            rho1 = const_pool.tile([2, V], bf16, tag="rho1")
            nc.sync.dma_start(out=rho1[:], in_=d_rho1[:])
            w12b = const_pool.tile([P, 4 * F], bf16, tag="w12b")
            nc.sync.dma_start(out=w12b[:], in_=d_w12b[:])
            w34 = const_pool.tile([P, 4 * F], bf16, tag="w34")
            nc.sync.dma_start(out=w34[:], in_=d_w34[:])
            w56 = const_pool.tile([P, 12], bf16, tag="w56")
            nc.sync.dma_start(out=w56[:], in_=d_w56[:])
            vvm = const_pool.tile([P, NV * G], bf16, tag="vvm")
            for uc in range(NV):
                nc.sync.dma_start(
                    out=vvm[:, uc * G:(uc + 1) * G], in_=d_vvm[uc]
                )
            ident = const_pool.tile([P, P], f32, tag="ident")
            make_identity(nc, ident[:])
            # dummy transpose: absorbs the gpsimd(identity) wait on the PE
            # clock -- walrus allows only ONE sync wait on transpose-mode
            # matmuls (S3 LW struct), so later transposes must carry only
            # their data dependency.
            ps_warm = psA.tile([1, P], f32, tag="psA")
            nc.tensor.transpose(
                out=ps_warm[:], in_=ident[:, 0:1], identity=ident[:]
            )
            warm_sink = const_pool.tile([1, P], f32, tag="warm")
            nc.vector.tensor_copy(out=warm_sink[:], in_=ps_warm[:])
            if has_bias2:
                biasp2 = const_pool.tile([2, F], bf16, tag="biasp2")
                nc.sync.dma_start(out=biasp2[:], in_=d_biasp2[:])
            if has_bias3:
                biasp3 = const_pool.tile([2, BL * 3], bf16, tag="biasp3")
                nc.sync.dma_start(out=biasp3[:], in_=d_biasp3[:])

            # ---------- phase 0: verts aggregation  av = A^2 @ verts ----------
            av_sb = const_pool.tile([P, NV * G], f32, tag="av")
            for dc in range(NV):
                ps = psA.tile([P, G], f32, tag="psA")
                for uc in range(NV):
                    nc.tensor.matmul(
                        out=ps[:],
                        lhsT=a2t[uc][:, dc * P:(dc + 1) * P],
                        rhs=vvm[:, uc * G:(uc + 1) * G],
                        start=(uc == 0),
                        stop=(uc == NV - 1),
                    )
                nc.vector.tensor_copy(
                    out=av_sb[:, dc * G:(dc + 1) * G], in_=ps[:]
                )

            # transpose av -> feature-major rows per batch, build av_aug
            # [3, V] = (A^2 verts_b)^T   (single producer engine: DVE)
            av_aug = []
            for b in range(BL):
                av_aug_b = const_pool.tile([3, V], bf16, tag=f"avaug{b}")
                av_aug.append(av_aug_b)
            for dc in range(NV):
                for b in range(BL):
                    ps = psA.tile([3, P], f32, tag="psA")
                    nc.tensor.transpose(
                        out=ps[:],
                        in_=av_sb[:, dc * G + b * 3: dc * G + (b + 1) * 3],
                        identity=ident[:],
                    )
                    nc.vector.tensor_copy(
                        out=av_aug[b][0:3, dc * P:(dc + 1) * P],
                        in_=ps[:],
                    )

            # t3 storage across batches: [128, (uc, b, cc)] bf16
            t3_bf = const_pool.tile([P, NV * G], bf16, tag="t3bf")

            # ---------- per batch: pair1 -> pair2 -> t3 ----------
            for b in range(BL):
                # c1 = img_b @ W12b (rank-1 image term) -> [1, 512] psum
                img_sb = work_pool.tile([P, 4], bf16, tag="img")
                nc.sync.dma_start(
                    out=img_sb[:],
                    in_=d_img[b].rearrange("(c p) -> p c", p=P),
                )
                ps_c1 = psA.tile([1, F], f32, tag="psA")
                for kc in range(4):
                    nc.tensor.matmul(
                        out=ps_c1[:],
                        lhsT=img_sb[:, kc:kc + 1],
                        rhs=w12b[:, kc * F:(kc + 1) * F],
                        start=(kc == 0),
                        stop=(kc == 3),
                    )
                c1_sb = work_pool.tile([1, F], bf16, tag="c1")
                nc.vector.tensor_copy(out=c1_sb[:], in_=ps_c1[:])

                # x1 feature-major [f, v] = relu(k=5 static matmul
                #   + k=1 image-term matmul), bf16
                x1 = []
                for fc in range(NF):
                    xt = x_pool.tile([P, V], bf16, tag="x")
                    for nh in range(2):
                        ps = psA.tile([P, 1024], f32, tag="psA")
                        for n2 in range(2):
                            col = (nh * 2 + n2) * 512
                            nc.tensor.matmul(
                                out=ps[:, n2 * 512:(n2 + 1) * 512],
                                lhsT=w12a_base[:, fc * P:(fc + 1) * P],
                                rhs=av_aug[b][:, col:col + 512],
                                start=True,
                                stop=False,
                            )
                            if has_bias1:
                                nc.tensor.matmul(
                                    out=ps[:, n2 * 512:(n2 + 1) * 512],
                                    lhsT=biasp1[:, fc * P:(fc + 1) * P],
                                    rhs=rho1[:, col:col + 512],
                                    start=False,
                                    stop=False,
                                )
                            nc.tensor.matmul(
                                out=ps[:, n2 * 512:(n2 + 1) * 512],
                                lhsT=c1_sb[:, fc * P:(fc + 1) * P],
                                rhs=rho2_sb[:, col:col + 512],
                                start=False,
                                stop=True,
                            )
                        nc.vector.tensor_scalar_max(
                            out=xt[:, nh * 1024:(nh + 1) * 1024],
                            in0=ps[:],
                            scalar1=0.0,
                        )
                    x1.append(xt)

                # t2 vertex-major bf16 [v, f]
                t2_bf = tbf_pool.tile([P, NV * F], bf16, tag="t2")
                for vc in range(NV):
                    ps = psB.tile([P, F], f32, tag="psB")
                    for kc in range(NF):
                        nc.tensor.matmul(
                            out=ps[:],
                            lhsT=x1[kc][:, vc * P:(vc + 1) * P],
                            rhs=w34[:, kc * F:(kc + 1) * F],
                            start=(kc == 0),
                            stop=(kc == NF - 1),
                        )
                    nc.vector.tensor_copy(
                        out=t2_bf[:, vc * F:(vc + 1) * F], in_=ps[:]
                    )

                # z2 = A^2 t2 (feature-major out) ; x2 = relu(z2) bf16
                x2 = []
                for fc in range(NF):
                    xt = x_pool.tile([P, V], bf16, tag="x")
                    for nh in range(2):
                        ps = psA.tile([P, 1024], f32, tag="psA")
                        for uc in range(NV):
                            for n2 in range(2):
                                col = nh * 1024 + n2 * 512
                                nc.tensor.matmul(
                                    out=ps[:, n2 * 512:(n2 + 1) * 512],
                                    lhsT=t2_bf[
                                        :, uc * F + fc * P:uc * F + (fc + 1) * P
                                    ],
                                    rhs=a2t[uc][:, col:col + 512],
                                    start=(uc == 0),
                                    stop=(uc == NV - 1 and not has_bias2),
                                )
                        if has_bias2:
                            for n2 in range(2):
                                col = nh * 1024 + n2 * 512
                                nc.tensor.matmul(
                                    out=ps[:, n2 * 512:(n2 + 1) * 512],
                                    lhsT=biasp2[:, fc * P:(fc + 1) * P],
                                    rhs=rho1[:, col:col + 512],
                                    start=False,
                                    stop=True,
                                )
                        nc.vector.tensor_scalar_max(
                            out=xt[:, nh * 1024:(nh + 1) * 1024],
                            in0=ps[:],
                            scalar1=0.0,
                        )
                    x2.append(xt)

                # t3 = x2 @ W56 -> [v, 3] vert-major, bf16 (b-interleaved)
                for vc in range(NV):
                    ps = psB.tile([P, 3], f32, tag="psB")
                    for kc in range(NF):
                        nc.tensor.matmul(
                            out=ps[:],
                            lhsT=x2[kc][:, vc * P:(vc + 1) * P],
                            rhs=w56[:, kc * 3:(kc + 1) * 3],
                            start=(kc == 0),
                            stop=(kc == NF - 1),
                        )
                    nc.vector.tensor_copy(
                        out=t3_bf[:, vc * G + b * 3: vc * G + (b + 1) * 3],
                        in_=ps[:],
                    )

            # ---------- pair3 aggregation, all batches ----------
            # x3_bf [128, (dc, cc, b)] bf16  (cc-major within dc for FC lhsT)
            x3_bf = const_pool.tile([P, NV * G], bf16, tag="x3bf")
            for dc in range(NV):
                ps = psA.tile([P, G], f32, tag="psA")
                for uc in range(NV):
                    nc.tensor.matmul(
                        out=ps[:],
                        lhsT=a2t[uc][:, dc * P:(dc + 1) * P],
                        rhs=t3_bf[:, uc * G:(uc + 1) * G],
                        start=(uc == 0),
                        stop=(uc == NV - 1 and not has_bias3),
                    )
                if has_bias3:
                    nc.tensor.matmul(
                        out=ps[:],
                        lhsT=rho1[:, dc * P:(dc + 1) * P],
                        rhs=biasp3[:],
                        start=False,
                        stop=True,
                    )
                # relu + permute (b,cc) -> (cc,b): FC k-chunk slices contiguous
                nc.vector.tensor_scalar_max(
                    out=x3_bf[:, dc * G:(dc + 1) * G]
                    .rearrange("p (c b) -> p b c", b=BL),
                    in0=ps[:].rearrange("p (b c) -> p b c", c=3),
                    scalar1=0.0,
                )

            # ---------- FC head (all batches together, bf16) ----------
            ps_h1 = psA.tile([BL, FC_H], f32, tag="psA")
            for kc in range(NKFC1):
                wt = stream_pool.tile([P, FC_H], bf16, tag="fcw")
                nc.sync.dma_start(out=wt[:], in_=d_fcw1[kc * P:(kc + 1) * P, :])
                vc, cc = divmod(kc, 3)
                lhsT = x3_bf[:, vc * G + cc * BL: vc * G + (cc + 1) * BL]
                for n2 in range(2):
                    nc.tensor.matmul(
                        out=ps_h1[:, n2 * 512:(n2 + 1) * 512],
                        lhsT=lhsT,
                        rhs=wt[:, n2 * 512:(n2 + 1) * 512],
                        start=(kc == 0),
                        stop=(kc == NKFC1 - 1),
                    )
            h1 = hfin_pool.tile([BL, FC_H], f32, tag="hfin")
            if has_fcb:
                fcb1_sb = hfin_pool.tile([BL, FC_H], f32, tag="hfin")
                nc.sync.dma_start(out=fcb1_sb[:], in_=d_fcb1[:])
                nc.vector.tensor_add(out=h1[:], in0=ps_h1[:], in1=fcb1_sb[:])
            else:
                nc.vector.tensor_copy(out=h1[:], in_=ps_h1[:])

            # transpose h1 -> h1T bf16 [128, (kc, b)]
            h1T = const_pool.tile([P, NKFC2 * BL], bf16, tag="h1T")
            for kc in range(NKFC2):
                ps = psA.tile([P, BL], f32, tag="psA")
                nc.tensor.transpose(
                    out=ps[:],
                    in_=h1[:, kc * P:(kc + 1) * P],
                    identity=ident[:BL, :BL],
                )
                nc.vector.tensor_copy(
                    out=h1T[:, kc * BL:(kc + 1) * BL], in_=ps[:]
                )

            ps_h2 = psA.tile([BL, FC_H], f32, tag="psA")
            for kc in range(NKFC2):
                wt = stream_pool.tile([P, FC_H], bf16, tag="fcw")
                nc.sync.dma_start(out=wt[:], in_=d_fcw2[kc * P:(kc + 1) * P, :])
                for n2 in range(2):
                    nc.tensor.matmul(
                        out=ps_h2[:, n2 * 512:(n2 + 1) * 512],
                        lhsT=h1T[:, kc * BL:(kc + 1) * BL],
                        rhs=wt[:, n2 * 512:(n2 + 1) * 512],
                        start=(kc == 0),
                        stop=(kc == NKFC2 - 1),
                    )
            h2 = hfin_pool.tile([BL, FC_H], f32, tag="hfin")
            if has_fcb:
                fcb2_sb = hfin_pool.tile([BL, FC_H], f32, tag="hfin")
                nc.sync.dma_start(out=fcb2_sb[:], in_=d_fcb2[:])
                nc.vector.tensor_add(out=h2[:], in0=ps_h2[:], in1=fcb2_sb[:])
            else:
                nc.vector.tensor_copy(out=h2[:], in_=ps_h2[:])

            h2T = const_pool.tile([P, NKFC2 * BL], bf16, tag="h2T")
            for kc in range(NKFC2):
                ps = psA.tile([P, BL], f32, tag="psA")
                nc.tensor.transpose(
                    out=ps[:],
                    in_=h2[:, kc * P:(kc + 1) * P],
                    identity=ident[:BL, :BL],
                )
                nc.vector.tensor_copy(
                    out=h2T[:, kc * BL:(kc + 1) * BL], in_=ps[:]
                )

            # FC3 + tanh + final add, in 1024-col chunks
            for ch in range(FLAT // FC_H):  # 6
                ps = psA.tile([BL, FC_H], f32, tag="psA")
                for kc in range(NKFC2):
                    wt = stream_pool.tile([P, FC_H], bf16, tag="fcw")
                    nc.sync.dma_start(
                        out=wt[:],
                        in_=d_fcw3[kc * P:(kc + 1) * P, ch * FC_H:(ch + 1) * FC_H],
                    )
                    for n2 in range(2):
                        nc.tensor.matmul(
                            out=ps[:, n2 * 512:(n2 + 1) * 512],
                            lhsT=h2T[:, kc * BL:(kc + 1) * BL],
                            rhs=wt[:, n2 * 512:(n2 + 1) * 512],
                            start=(kc == 0),
                            stop=(kc == NKFC2 - 1),
                        )
                d = hfin_pool.tile([BL, FC_H], f32, tag="hfin")
                if has_fcb:
                    fcb3_sb = hfin_pool.tile([BL, FC_H], f32, tag="hfin")
                    nc.sync.dma_start(
                        out=fcb3_sb[:], in_=d_fcb3[:, ch * FC_H:(ch + 1) * FC_H]
                    )
                    nc.vector.tensor_add(out=d[:], in0=ps[:], in1=fcb3_sb[:])
                    nc.scalar.activation(out=d[:], in_=d[:], func=AF.Tanh)
                else:
                    nc.scalar.activation(out=d[:], in_=ps[:], func=AF.Tanh)
                vf = hfin_pool.tile([BL, FC_H], f32, tag="hfin")
                nc.sync.dma_start(
                    out=vf[:], in_=d_vflat[:, ch * FC_H:(ch + 1) * FC_H]
                )
                o = hfin_pool.tile([BL, FC_H], f32, tag="hfin")
                nc.vector.tensor_scalar_mul(out=o[:], in0=d[:], scalar1=0.1)
                nc.vector.tensor_add(out=o[:], in0=o[:], in1=vf[:])
                nc.sync.dma_start(
                    out=d_out[:, ch * FC_H:(ch + 1) * FC_H], in_=o[:]
                )

    nc.finalize()
    return nc


def build_in_maps(inputs):
    """Host prep + per-core input maps (exposed for testing)."""
    shared, per_core = _host_prep(inputs)
    key = (shared["HAS_BIAS1"], shared["HAS_BIAS2"], shared["HAS_BIAS3"],
           shared["HAS_FCB"])
    shared_arrays = {k: v for k, v in shared.items() if isinstance(v, np.ndarray)}
    in_maps = []
    for c in range(N_CORES):
        m = dict(shared_arrays)
        m.update(per_core[c])
        in_maps.append(m)
    return key, in_maps


def kernel(**inputs):
    key, in_maps = build_in_maps(inputs)
    if key not in _CACHE:
        _CACHE[key] = _build_program(*key)
    nc = _CACHE[key]

    from concourse.bass_utils import run_bass_kernel_spmd

    res = run_bass_kernel_spmd(nc, in_maps, list(range(N_CORES)))
    out = np.empty((B, V, 3), np.float32)
    for c in range(N_CORES):
        out[c * BL:(c + 1) * BL] = np.asarray(
            res.results[c]["OUT"], np.float32
        ).reshape(BL, V, 3)
    return out
